# revision 3
# baseline (speedup 1.0000x reference)
"""Trainium2 Bass kernel for nn_CellNetwork — one-hot-matmul segment sums.

Redesign vs baseline: all segment-sums (CWNN Ldo/Lup spmm, GNN agg, final
dual scatter) run as dma_gather (HBM table -> SBUF msgs in [token, D]
layout) followed by PE matmuls against one-hot selection matrices S built
on DVE (tensor_scalar is_equal vs iota), accumulating in PSUM. This removes
every dma_scatter_add (the baseline's Pool-engine bottleneck: ~10.45us of
in-order Pool occupancy per 1024-token call).

Token streams are host-sorted per rank. SPMD requires an identical program
on all 8 cores, so every (window, chunk) run is padded to the max count
over ranks (pads: gather row 0, sidx=-1 -> S column all zero, val=0).
CWNN groups are (window, 1024-cell dchunk)-pure; GNN (512-node chunk)-pure
(single window); final (window, 512-node chunk)-pure.
"""
import sys
import numpy as np

sys.path.insert(0, "/opt/trn_rl_repo")

N = 20000
E = 200000
D = 128
NNZ = 400000
L = 3
NCORES = 8
RN = N // NCORES          # 2500 nodes per rank
RE = E // NCORES          # 25000 cells per rank
REP = 25088               # cells per rank, padded (196*128)
RNP = 2560                # nodes per rank, padded (20*128)
EP_TBL = NCORES * REP     # 200704-row padded cell table
NP_TBL = NCORES * RNP     # 20480-row padded node table
WIN = 32768               # gather window (int16 idx limit)
NW_E = (EP_TBL + WIN - 1) // WIN   # 7 windows over the cell table
NCH_E = REP // 512        # 49 chunks of 512 cells
NDCH = (NCH_E + 1) // 2   # 25 dchunks (24 full 1024 + 1 half)
NCH_N = RNP // 512        # 5 node chunks
MAXTOK = 1024


def _pad128(n):
    return (n + 127) & ~127


class _Prep:
    pass


# ---------------------------------------------------------------------------
# Host-side stream building
# ---------------------------------------------------------------------------

def _build_streams(per_rank_tokens, stream_keys, run_key_of, run_order):
    """Generic SPMD stream builder.

    per_rank_tokens: list over ranks of dict stream_key -> dict run_key ->
        (gidx_win_rel, sidx_local, val or None) arrays.
    stream_keys: ordered list of stream keys.
    run_order: dict stream_key -> ordered list of run keys.
    Returns (layout, fills):
      layout: per stream: dict(slot0, nslots, calls=[(w, lslot0, ntok)],
              groups=[(lgslot, run_key)])
      fills: per rank: (gidx[int16 slots], sidx[f32 slots], val[f16 slots])
    """
    nranks = len(per_rank_tokens)
    layout = {}
    slot0 = 0
    for sk in stream_keys:
        runs = []
        for rk in run_order[sk]:
            mx = 0
            for r in range(nranks):
                t = per_rank_tokens[r].get(sk, {}).get(rk)
                if t is not None:
                    mx = max(mx, len(t[0]))
            if mx:
                runs.append((rk, _pad128(mx)))
        calls = []
        groups = []
        ls = 0
        cur_w, cur_s0, cur_n = None, 0, 0
        for rk, nsl in runs:
            w = rk[0]
            for g in range(nsl // 128):
                groups.append((ls // 128 + g, rk))
            o = 0
            while o < nsl:
                take = min(nsl - o, MAXTOK - cur_n) if cur_w == w else 0
                if cur_w != w or take == 0:
                    if cur_n:
                        calls.append((cur_w, cur_s0, cur_n))
                    cur_w, cur_s0, cur_n = w, ls + o, 0
                    take = min(nsl - o, MAXTOK)
                cur_n += take
                o += take
                if cur_n == MAXTOK:
                    calls.append((cur_w, cur_s0, cur_n))
                    cur_w, cur_n = None, 0
            ls += nsl
        if cur_n:
            calls.append((cur_w, cur_s0, cur_n))
        nslots = sum(n for _, n in runs)
        layout[sk] = dict(slot0=slot0, nslots=nslots, calls=calls,
                          groups=groups, runs=runs)
        slot0 += nslots
    total = slot0
    fills = []
    for r in range(nranks):
        gi = np.zeros(total, np.int16)
        si = np.full(total, -1.0, np.float32)
        va = np.zeros(total, np.float16)
        for sk in stream_keys:
            lay = layout[sk]
            o = lay["slot0"]
            for rk, nsl in lay["runs"]:
                t = per_rank_tokens[r].get(sk, {}).get(rk)
                if t is not None:
                    n = len(t[0])
                    gi[o:o + n] = t[0].astype(np.int16)
                    si[o:o + n] = t[1].astype(np.float32)
                    if t[2] is not None:
                        va[o:o + n] = t[2].astype(np.float16)
                o += nsl
        fills.append((gi, si, va))
    return layout, total, fills


def _tok_tiles(total, fills):
    """Pack per-rank slot arrays into device tiles.

    gather pack [16, total/16] i16 (token t at (t%16, t//16));
    sidx [128, total/128] f32, val [128, total/128] f16 (t at (t%128, t//128)).
    """
    t_ = np.arange(total)
    packs, sids, vals = [], [], []
    for gi, si, va in fills:
        pk = np.zeros((16, total // 16), np.int16)
        pk[t_ % 16, t_ // 16] = gi
        sd = np.zeros((128, total // 128), np.float32)
        sd[t_ % 128, t_ // 128] = si
        vl = np.zeros((128, total // 128), np.float16)
        vl[t_ % 128, t_ // 128] = va
        packs.append(pk)
        sids.append(sd)
        vals.append(vl)
    return packs, sids, vals


def _preprocess(x, xe, gnn_W, cwnn_W0, cwnn_W1, cwnn_W2, Ldo_val, Lup_val,
                edges, row, col, Ldo_idx, Lup_idx):
    p = _Prep()
    f16 = np.float16
    x16 = np.asarray(x, np.float32).astype(f16)
    xe16 = np.asarray(xe, np.float32).astype(f16)
    x_pad = np.zeros((NP_TBL, D), f16)
    x_pad.reshape(NCORES, RNP, D)[:, :RN] = x16.reshape(NCORES, RN, D)
    xe_pad = np.zeros((EP_TBL, D), f16)
    xe_pad.reshape(NCORES, REP, D)[:, :RE] = xe16.reshape(NCORES, RE, D)
    p.xT = [np.ascontiguousarray(x_pad[r * RNP:(r + 1) * RNP].T)
            for r in range(NCORES)]
    p.xeT = [np.ascontiguousarray(xe_pad[r * REP:(r + 1) * REP].T)
             for r in range(NCORES)]
    p.gnnW = np.ascontiguousarray(
        np.asarray(gnn_W, np.float32).astype(f16).transpose(1, 0, 2).reshape(D, L * D))
    p.W0 = np.ascontiguousarray(
        np.asarray(cwnn_W0, np.float32).astype(f16).transpose(1, 0, 2).reshape(D, L * D))
    p.W1 = np.ascontiguousarray(
        np.asarray(cwnn_W1, np.float32).astype(f16).transpose(1, 0, 2).reshape(D, L * D))
    p.W2 = np.ascontiguousarray(
        np.asarray(cwnn_W2, np.float32).astype(f16).transpose(1, 0, 2).reshape(D, L * D))
    p.ident = np.eye(D, dtype=f16)
    p.iota = np.broadcast_to(np.arange(1024, dtype=f16), (128, 1024)).copy()

    edges = np.asarray(edges, np.int64)
    row = np.asarray(row, np.int64)
    col = np.asarray(col, np.int64)
    Ldo_idx = np.asarray(Ldo_idx, np.int64)
    Lup_idx = np.asarray(Lup_idx, np.int64)

    # ---------------- CWNN streams: key (split, lap), runs (w, dchunk) ----
    # splits partition the 25 dchunks for SBUF msg-buffer streaming
    SPL = [list(range(a, min(a + 3, NDCH))) for a in range(0, NDCH, 3)]
    p.cw_splits = SPL
    dch_of = {}
    for si_, ds in enumerate(SPL):
        for d_ in ds:
            dch_of[d_] = si_
    lap_data = [(Ldo_idx, np.asarray(Ldo_val, np.float32)),
                (Lup_idx, np.asarray(Lup_val, np.float32))]
    per_rank = [dict() for _ in range(NCORES)]
    for lap, (lidx, lval) in enumerate(lap_data):
        rr = lidx[0] // RE
        for r in range(NCORES):
            sel = rr == r
            lr = lidx[0][sel] - r * RE
            d_ = lr // 1024
            cr = lr % 1024
            gc = lidx[1][sel]
            gp = (gc // RE) * REP + gc % RE
            w = gp // WIN
            wi = gp % WIN
            v = lval[sel]
            sp = d_ // 3
            key = ((sp * 2 + 0) * NW_E + w) * NDCH + d_
            order = np.argsort(key, kind="stable")
            wi, cr, v, w, d_, sp = (wi[order], cr[order], v[order], w[order],
                                    d_[order], sp[order])
            uq, first, cnt = np.unique(
                ((sp * NW_E + w) * NDCH + d_), return_index=True,
                return_counts=True)
            for u, f0, c_ in zip(uq, first, cnt):
                dd = int(u) % NDCH
                ww = (int(u) // NDCH) % NW_E
                ss = int(u) // (NDCH * NW_E)
                sk = (ss, lap)
                rk = (ww, dd)
                per_rank[r].setdefault(sk, {})[rk] = (
                    wi[f0:f0 + c_], cr[f0:f0 + c_], v[f0:f0 + c_])
    stream_keys = []
    for si_ in range(len(SPL)):
        for lap in range(2):
            stream_keys.append((si_, lap))
    run_order = {}
    for sk in stream_keys:
        si_, lap = sk
        run_order[sk] = [(w, d_) for w in range(NW_E) for d_ in SPL[si_]]
    p.cw_layout, p.cw_slots, cw_fills = _build_streams(
        per_rank, stream_keys, None, run_order)
    p.cw_stream_keys = stream_keys
    p.cw_pack, p.cw_sidx, p.cw_val = _tok_tiles(p.cw_slots, cw_fills)
    p.cw_gmax = max(l_["nslots"] for l_ in p.cw_layout.values()) // 128

    # ---------------- GNN streams: key (chunk), runs (0, chunk) ----------
    src, dst = edges[0], edges[1]
    rr = dst // RN
    per_rank_g = [dict() for _ in range(NCORES)]
    for r in range(NCORES):
        sel = rr == r
        lr = dst[sel] - r * RN
        c_ = lr // 512
        cr = lr % 512
        gp = (src[sel] // RN) * RNP + src[sel] % RN
        order = np.argsort(c_, kind="stable")
        gp, cr, c_ = gp[order], cr[order], c_[order]
        for cc in range(NCH_N):
            m = c_ == cc
            if m.any():
                per_rank_g[r].setdefault((cc,), {})[(0, cc)] = (
                    gp[m], cr[m], None)
    g_keys = [(cc,) for cc in range(NCH_N)]
    g_ro = {(cc,): [(0, cc)] for cc in range(NCH_N)}
    p.g_layout, p.g_slots, g_fills = _build_streams(
        per_rank_g, g_keys, None, g_ro)
    p.g_stream_keys = g_keys
    p.g_pack, p.g_sidx, _ = _tok_tiles(p.g_slots, g_fills)
    p.g_gmax = max(l_["nslots"] for l_ in p.g_layout.values()) // 128

    # ---------------- final streams: key (node chunk), runs (w, cn) ------
    nodes = np.concatenate([row, col])
    cells = np.concatenate([np.arange(E), np.arange(E)])
    rr = nodes // RN
    per_rank_f = [dict() for _ in range(NCORES)]
    for r in range(NCORES):
        sel = rr == r
        lr = nodes[sel] - r * RN
        cn = lr // 512
        cr = lr % 512
        gp = (cells[sel] // RE) * REP + cells[sel] % RE
        w = gp // WIN
        wi = gp % WIN
        key = cn * NW_E + w
        order = np.argsort(key, kind="stable")
        wi, cr, cn, w = wi[order], cr[order], cn[order], w[order]
        uq, first, cnt = np.unique(cn * NW_E + w, return_index=True,
                                   return_counts=True)
        for u, f0, c_ in zip(uq, first, cnt):
            ww = int(u) % NW_E
            cc = int(u) // NW_E
            per_rank_f[r].setdefault((cc,), {})[(ww, cc)] = (
                wi[f0:f0 + c_], cr[f0:f0 + c_], None)
    f_keys = [(cc,) for cc in range(NCH_N)]
    f_ro = {(cc,): [(w, cc) for w in range(NW_E)] for cc in range(NCH_N)}
    p.f_layout, p.f_slots, f_fills = _build_streams(
        per_rank_f, f_keys, None, f_ro)
    p.f_stream_keys = f_keys
    p.f_pack, p.f_sidx, _ = _tok_tiles(p.f_slots, f_fills)
    p.f_gmax = max(l_["nslots"] for l_ in p.f_layout.values()) // 128

    p.gmax = max(p.cw_gmax, p.g_gmax, p.f_gmax)

    # ---------------- pack everything into one blob per rank -------------
    # column-concatenated [128, C_total] f16; i16 regions stored bit-cast,
    # f32 sidx stored as f16 values (exact: 0..1023 and -1), converted on
    # device with one tensor_copy.
    p.cw_pack_rep = [np.broadcast_to(
        pk.reshape(1, 16, -1), (8, 16, pk.shape[1])).reshape(128, -1)
        for pk in p.cw_pack]
    p.g_pack_rep = [np.broadcast_to(
        pk.reshape(1, 16, -1), (8, 16, pk.shape[1])).reshape(128, -1)
        for pk in p.g_pack]
    p.f_pack_rep = [np.broadcast_to(
        pk.reshape(1, 16, -1), (8, 16, pk.shape[1])).reshape(128, -1)
        for pk in p.f_pack]
    p.blob_layout = [
        ("xT", RNP), ("xeT", REP), ("gnnW", L * D), ("W0", L * D),
        ("W1", L * D), ("W2", L * D), ("ident", D), ("iota", 1024),
        ("cwpk", p.cw_slots // 16), ("cwsd", p.cw_slots // 128),
        ("cwvl", p.cw_slots // 128), ("gpk", p.g_slots // 16),
        ("gsd", p.g_slots // 128), ("fpk", p.f_slots // 16),
        ("fsd", p.f_slots // 128),
    ]
    p.blob_cols = sum(c for _, c in p.blob_layout)
    p.blob = []
    for r in range(NCORES):
        parts = {
            "xT": p.xT[r], "xeT": p.xeT[r], "gnnW": p.gnnW, "W0": p.W0,
            "W1": p.W1, "W2": p.W2, "ident": p.ident, "iota": p.iota,
            "cwpk": p.cw_pack_rep[r].view(np.float16),
            "cwsd": p.cw_sidx[r].astype(np.float16),
            "cwvl": p.cw_val[r],
            "gpk": p.g_pack_rep[r].view(np.float16),
            "gsd": p.g_sidx[r].astype(np.float16),
            "fpk": p.f_pack_rep[r].view(np.float16),
            "fsd": p.f_sidx[r].astype(np.float16),
        }
        b = np.zeros((128, p.blob_cols), np.float16)
        o = 0
        for nm, c in p.blob_layout:
            b[:, o:o + c] = parts[nm]
            o += c
        p.blob.append(b)

    p.skey = (p.cw_slots, p.g_slots, p.f_slots, p.gmax,
              tuple((sk, tuple(l_["calls"]), tuple(l_["groups"]))
                    for sk, l_ in p.cw_layout.items()),
              tuple((sk, tuple(l_["calls"]), tuple(l_["groups"]))
                    for sk, l_ in p.g_layout.items()),
              tuple((sk, tuple(l_["calls"]), tuple(l_["groups"]))
                    for sk, l_ in p.f_layout.items()))
    return p


# ---------------------------------------------------------------------------
# Device program
# ---------------------------------------------------------------------------

def _build(p, stage=99):
    import concourse.bacc as bacc
    import concourse.tile as tile
    import concourse.mybir as mybir
    F16 = mybir.dt.float16
    F32 = mybir.dt.float32
    I16 = mybir.dt.int16
    RELU = mybir.ActivationFunctionType.Relu
    COPY = mybir.ActivationFunctionType.Copy
    EQ = mybir.AluOpType.is_equal
    MUL = mybir.AluOpType.mult
    ADD = mybir.AluOpType.add

    nc = bacc.Bacc("TRN2", target_bir_lowering=False, debug=False,
                   num_devices=NCORES)

    blob = nc.dram_tensor("blob", [128, p.blob_cols], F16,
                          kind="ExternalInput")
    out = nc.dram_tensor("out", [RNP, 2 * D], F16, kind="ExternalOutput")
    boff = {}
    _o = 0
    for nm, c in p.blob_layout:
        boff[nm] = _o
        _o += c

    def bsl(nm, cols):
        return blob.ap()[:, boff[nm]:boff[nm] + cols]

    h_full = [nc.dram_tensor(f"h_full{i}", [NP_TBL, D], F16,
                             addr_space="Shared") for i in range(L)]
    he_full = [nc.dram_tensor(f"he_full{i}", [EP_TBL, D], F16,
                              addr_space="Shared") for i in range(L + 1)]
    h_bounce = [nc.dram_tensor(f"h_bounce{i}", [RNP, D], F16)
                for i in range(L)]
    he_bounce = [nc.dram_tensor(f"he_bounce{i}", [REP, D], F16)
                 for i in range(L + 1)]
    rg = [list(range(NCORES))]
    dbg = (nc.dram_tensor("dbg", [REP, D], F16, kind="ExternalOutput")
           if stage < 99 else None)

    def wrows_e(w):
        return min(WIN, EP_TBL - w * WIN)

    with tile.TileContext(nc) as tc:
        with tc.tile_pool(name="const", bufs=1) as cpool, \
             tc.tile_pool(name="msg", bufs=3) as mpool, \
             tc.tile_pool(name="S", bufs=4) as spool, \
             tc.tile_pool(name="st", bufs=6) as stpool, \
             tc.tile_pool(name="tt", bufs=4) as tpool, \
             tc.tile_pool(name="ps_seg", bufs=2, space="PSUM") as pseg, \
             tc.tile_pool(name="ps_w", bufs=2, space="PSUM") as psw, \
             tc.tile_pool(name="ps_t", bufs=2, space="PSUM") as pst:

            ident = cpool.tile([D, D], F16, tag="ident")
            nc.sync.dma_start(out=ident[:], in_=bsl("ident", D))
            iota = cpool.tile([128, 1024], F16, tag="iota")
            nc.sync.dma_start(out=iota[:], in_=bsl("iota", 1024))
            gW = cpool.tile([D, L * D], F16, tag="gW")
            nc.sync.dma_start(out=gW[:], in_=bsl("gnnW", L * D))
            w0 = cpool.tile([D, L * D], F16, tag="w0")
            nc.sync.dma_start(out=w0[:], in_=bsl("W0", L * D))
            w1 = cpool.tile([D, L * D], F16, tag="w1")
            nc.sync.dma_start(out=w1[:], in_=bsl("W1", L * D))
            w2 = cpool.tile([D, L * D], F16, tag="w2")
            nc.sync.dma_start(out=w2[:], in_=bsl("W2", L * D))
            hT = cpool.tile([D, RNP], F16, tag="hT")
            nc.sync.dma_start(out=hT[:], in_=bsl("xT", RNP))
            heT = cpool.tile([D, REP], F16, tag="heT")
            nc.sync.dma_start(out=heT[:], in_=bsl("xeT", REP))
            cwpk = cpool.tile([128, p.cw_slots // 16], I16, tag="cwpk")
            nc.sync.dma_start(out=cwpk[:],
                              in_=bsl("cwpk", p.cw_slots // 16).bitcast(I16))
            cwsd16 = cpool.tile([128, p.cw_slots // 128], F16, tag="cwsd16")
            nc.sync.dma_start(out=cwsd16[:],
                              in_=bsl("cwsd", p.cw_slots // 128))
            cwsd = cpool.tile([128, p.cw_slots // 128], F32, tag="cwsd")
            nc.vector.tensor_copy(cwsd[:], cwsd16[:])
            cwvl16 = cpool.tile([128, p.cw_slots // 128], F16, tag="cwvl16")
            nc.sync.dma_start(out=cwvl16[:],
                              in_=bsl("cwvl", p.cw_slots // 128))
            cwvl = cpool.tile([128, p.cw_slots // 128], F32, tag="cwvl")
            nc.vector.tensor_copy(cwvl[:], cwvl16[:])
            gpk = cpool.tile([128, p.g_slots // 16], I16, tag="gpk")
            nc.sync.dma_start(out=gpk[:],
                              in_=bsl("gpk", p.g_slots // 16).bitcast(I16))
            gsd16 = cpool.tile([128, p.g_slots // 128], F16, tag="gsd16")
            nc.sync.dma_start(out=gsd16[:],
                              in_=bsl("gsd", p.g_slots // 128))
            gsd = cpool.tile([128, p.g_slots // 128], F32, tag="gsd")
            nc.vector.tensor_copy(gsd[:], gsd16[:])
            fpk = cpool.tile([128, p.f_slots // 16], I16, tag="fpk")
            nc.sync.dma_start(out=fpk[:],
                              in_=bsl("fpk", p.f_slots // 16).bitcast(I16))
            fsd16 = cpool.tile([128, p.f_slots // 128], F16, tag="fsd16")
            nc.sync.dma_start(out=fsd16[:],
                              in_=bsl("fsd", p.f_slots // 128))
            fsd = cpool.tile([128, p.f_slots // 128], F32, tag="fsd")
            nc.vector.tensor_copy(fsd[:], fsd16[:])

            _regs = {}

            def _reg(v):
                if v not in _regs:
                    _regs[v] = nc.gpsimd.to_reg(v)
                return _regs[v]

            def bounce_ag(srcT, ncols, bounce_d, full_d):
                for c in range(ncols // 512):
                    c0 = c * 512
                    psb = pst.tile([128, 512], F32, tag="ps_t")
                    for k in range(4):
                        r0 = c0 + k * 128
                        nc.tensor.matmul(psb[:, k * 128:(k + 1) * 128],
                                         lhsT=srcT[:, r0:r0 + 128],
                                         rhs=ident[:], start=True, stop=True)
                    bt = tpool.tile([128, 512], F16, tag="bt")
                    nc.vector.tensor_copy(bt[:], psb[:])
                    for k in range(4):
                        r0 = c0 + k * 128
                        nc.sync.dma_start(
                            out=bounce_d.ap()[r0:r0 + 128, :],
                            in_=bt[:, k * 128:(k + 1) * 128])
                nc.gpsimd.collective_compute(
                    "AllGather", mybir.AluOpType.bypass, replica_groups=rg,
                    ins=[bounce_d.ap().opt()], outs=[full_d.ap().opt()])

            def transpose_to(srcT, c0, width, dst_dram, dst_r0):
                """rows c0..c0+width of row-major dst from srcT columns."""
                psb = pst.tile([128, 512], F32, tag="ps_t")
                for k in range(width // 128):
                    r0 = c0 + k * 128
                    nc.tensor.matmul(psb[:, k * 128:(k + 1) * 128],
                                     lhsT=srcT[:, r0:r0 + 128],
                                     rhs=ident[:], start=True, stop=True)
                bt = tpool.tile([128, 512], F16, tag="bt")
                nc.vector.tensor_copy(bt[:, :width], psb[:, :width])
                for k in range(width // 128):
                    nc.sync.dma_start(
                        out=dst_dram.ap()[dst_r0 + k * 128:
                                          dst_r0 + (k + 1) * 128, :],
                        in_=bt[:, k * 128:(k + 1) * 128])

            def gather_stream(lay, pack_t, table, wrows, msg):
                """Issue the gather calls of one stream into msg tile."""
                s0 = lay["slot0"]
                for (w, ls0, ntok) in lay["calls"]:
                    a = s0 + ls0
                    nc.gpsimd.dma_gather(
                        msg[:, ls0 // 128:(ls0 + ntok) // 128, :],
                        table.ap()[w * WIN:w * WIN + wrows(w), :],
                        pack_t[:, a // 16:(a + ntok) // 16],
                        ntok, _reg(ntok), D)

            # ============ build level-0 gather tables ============
            bounce_ag(heT, REP, he_bounce[L], he_full[0])
            bounce_ag(hT, RNP, h_bounce[L - 1], h_full[0])

            # ============ layers ============
            for i in range(L):
                # ---------- CWNN layer i ----------
                he_tab = he_full[i]
                for si_, ds in enumerate(p.cw_splits):
                    msgs = {}
                    for lap in range(2):
                        lay = p.cw_layout[(si_, lap)]
                        m = mpool.tile([128, p.gmax, D], F16, tag="msg")
                        gather_stream(lay, cwpk, he_tab, wrows_e, m)
                        msgs[lap] = m
                    for d_ in ds:
                        wd = 1024 if 2 * d_ + 1 < NCH_E else 512
                        c0 = d_ * 1024
                        stg = {}
                        for lap in range(2):
                            lay = p.cw_layout[(si_, lap)]
                            gs = [g for g in lay["groups"] if g[1][1] == d_]
                            gb = lay["slot0"] // 128
                            psA = pseg.tile([128, 512], F32, tag="segA")
                            psB = (pseg.tile([128, 512], F32, tag="segB",
                                             name="psB")
                                   if wd == 1024 else None)
                            for k, (lg, rk) in enumerate(gs):
                                S = spool.tile([128, 1024], F16, tag="S")
                                nc.vector.tensor_scalar(
                                    out=S[:, :wd], in0=iota[:, :wd],
                                    scalar1=cwsd[:, gb + lg:gb + lg + 1],
                                    scalar2=cwvl[:, gb + lg:gb + lg + 1],
                                    op0=EQ, op1=MUL)
                                nc.tensor.matmul(
                                    psA[:], lhsT=msgs[lap][:, lg, :],
                                    rhs=S[:, :512], start=(k == 0),
                                    stop=(k == len(gs) - 1))
                                if wd == 1024:
                                    nc.tensor.matmul(
                                        psB[:], lhsT=msgs[lap][:, lg, :],
                                        rhs=S[:, 512:1024], start=(k == 0),
                                        stop=(k == len(gs) - 1))
                            for half in range(wd // 512):
                                ps_ = psA if half == 0 else psB
                                st = stpool.tile([128, 512], F16, tag="st")
                                if gs:
                                    if lap == 0:
                                        nc.vector.tensor_copy(st[:], ps_[:])
                                    else:
                                        nc.scalar.activation(st[:], ps_[:],
                                                             COPY)
                                else:
                                    nc.vector.memset(st[:], 0.0)
                                stg[(lap, half)] = st
                        for half in range(wd // 512):
                            cc0 = c0 + half * 512
                            ps = psw.tile([128, 512], F32, tag="ps_w")
                            nc.tensor.matmul(ps[:],
                                             lhsT=w0[:, i * D:(i + 1) * D],
                                             rhs=heT[:, cc0:cc0 + 512],
                                             start=True, stop=False)
                            nc.tensor.matmul(ps[:],
                                             lhsT=w1[:, i * D:(i + 1) * D],
                                             rhs=stg[(0, half)][:],
                                             start=False, stop=False)
                            nc.tensor.matmul(ps[:],
                                             lhsT=w2[:, i * D:(i + 1) * D],
                                             rhs=stg[(1, half)][:],
                                             start=False, stop=True)
                            nc.scalar.activation(heT[:, cc0:cc0 + 512],
                                                 ps[:], RELU)
                            transpose_to(heT, cc0, 512, he_bounce[i], cc0)

                if stage == 1 and i == 0:
                    nc.sync.dma_start(out=dbg.ap(), in_=he_bounce[0].ap())
                    break

                nc.gpsimd.collective_compute(
                    "AllGather", mybir.AluOpType.bypass, replica_groups=rg,
                    ins=[he_bounce[i].ap().opt()],
                    outs=[he_full[i + 1].ap().opt()])

                # ---------- GNN layer i ----------
                h_tab = h_full[i]
                for cc in range(NCH_N):
                    lay = p.g_layout[(cc,)]
                    m = mpool.tile([128, p.gmax, D], F16, tag="msg")
                    gather_stream(lay, gpk, h_tab,
                                  lambda w: NP_TBL, m)
                    gb = lay["slot0"] // 128
                    gs = lay["groups"]
                    c0 = cc * 512
                    psA = pseg.tile([128, 512], F32, tag="segA")
                    for k, (lg, rk) in enumerate(gs):
                        S = spool.tile([128, 1024], F16, tag="S")
                        nc.vector.tensor_scalar(
                            out=S[:, :512], in0=iota[:, :512],
                            scalar1=gsd[:, gb + lg:gb + lg + 1],
                            scalar2=None, op0=EQ)
                        nc.tensor.matmul(
                            psA[:], lhsT=m[:, lg, :], rhs=S[:, :512],
                            start=(k == 0), stop=(k == len(gs) - 1))
                    st = stpool.tile([128, 512], F16, tag="st")
                    if gs:
                        nc.vector.tensor_tensor(
                            out=st[:], in0=psA[:], in1=hT[:, c0:c0 + 512],
                            op=ADD)
                    else:
                        nc.vector.tensor_copy(st[:], hT[:, c0:c0 + 512])
                    ps = psw.tile([128, 512], F32, tag="ps_w")
                    nc.tensor.matmul(ps[:], lhsT=gW[:, i * D:(i + 1) * D],
                                     rhs=st[:], start=True, stop=True)
                    nc.scalar.activation(hT[:, c0:c0 + 512], ps[:], RELU)
                    if i < L - 1:
                        transpose_to(hT, c0, 512, h_bounce[i], c0)
                if i < L - 1:
                    nc.gpsimd.collective_compute(
                        "AllGather", mybir.AluOpType.bypass,
                        replica_groups=rg,
                        ins=[h_bounce[i].ap().opt()],
                        outs=[h_full[i + 1].ap().opt()])

            # ============ final: dual scatter of he3 + h3 out ============
            if stage >= 99:
                for cc in range(NCH_N):
                    lay = p.f_layout[(cc,)]
                    m = mpool.tile([128, p.gmax, D], F16, tag="msg")
                    gather_stream(lay, fpk, he_full[L], wrows_e, m)
                    gb = lay["slot0"] // 128
                    gs = lay["groups"]
                    c0 = cc * 512
                    psA = pseg.tile([128, 512], F32, tag="segA")
                    for k, (lg, rk) in enumerate(gs):
                        S = spool.tile([128, 1024], F16, tag="S")
                        nc.vector.tensor_scalar(
                            out=S[:, :512], in0=iota[:, :512],
                            scalar1=fsd[:, gb + lg:gb + lg + 1],
                            scalar2=None, op0=EQ)
                        nc.tensor.matmul(
                            psA[:], lhsT=m[:, lg, :], rhs=S[:, :512],
                            start=(k == 0), stop=(k == len(gs) - 1))
                    xst = stpool.tile([128, 512], F16, tag="st")
                    if gs:
                        nc.vector.tensor_copy(xst[:], psA[:])
                    else:
                        nc.vector.memset(xst[:], 0.0)
                    # transpose xedT chunk -> out[:, D:2D]
                    psb = pst.tile([128, 512], F32, tag="ps_t")
                    for k in range(4):
                        nc.tensor.matmul(psb[:, k * 128:(k + 1) * 128],
                                         lhsT=xst[:, k * 128:(k + 1) * 128],
                                         rhs=ident[:], start=True, stop=True)
                    bt = tpool.tile([128, 512], F16, tag="bt")
                    nc.vector.tensor_copy(bt[:], psb[:])
                    for k in range(4):
                        nc.sync.dma_start(
                            out=out.ap()[c0 + k * 128:c0 + (k + 1) * 128,
                                         D:2 * D],
                            in_=bt[:, k * 128:(k + 1) * 128])
                    # h3 part -> out[:, 0:D]
                    psb2 = pst.tile([128, 512], F32, tag="ps_t")
                    for k in range(4):
                        r0 = c0 + k * 128
                        nc.tensor.matmul(psb2[:, k * 128:(k + 1) * 128],
                                         lhsT=hT[:, r0:r0 + 128],
                                         rhs=ident[:], start=True, stop=True)
                    bt2 = tpool.tile([128, 512], F16, tag="bt")
                    nc.vector.tensor_copy(bt2[:], psb2[:])
                    for k in range(4):
                        nc.sync.dma_start(
                            out=out.ap()[c0 + k * 128:c0 + (k + 1) * 128,
                                         0:D],
                            in_=bt2[:, k * 128:(k + 1) * 128])

    nc.compile()
    return nc


# ---------------------------------------------------------------------------
# PJRT runner (same as baseline)
# ---------------------------------------------------------------------------

def _make_runner(nc):
    import jax
    import time
    from jax.sharding import Mesh, PartitionSpec
    from jax.experimental.shard_map import shard_map
    import concourse.mybir as mybir
    import concourse.bass2jax as bass2jax
    from concourse.bass2jax import _bass_exec_p, install_neuronx_cc_hook

    install_neuronx_cc_hook()
    partition_name = (nc.partition_id_tensor.name
                      if nc.partition_id_tensor else None)

    in_names, out_names, out_avals, zero_outs = [], [], [], []
    for alloc in nc.m.functions[0].allocations:
        if not isinstance(alloc, mybir.MemoryLocationSet):
            continue
        name = alloc.memorylocations[0].name
        if alloc.kind == "ExternalInput":
            if name != partition_name:
                in_names.append(name)
        elif alloc.kind == "ExternalOutput":
            out_names.append(name)
            shape = tuple(alloc.tensor_shape)
            dtype = mybir.dt.np(alloc.dtype)
            out_avals.append(jax.core.ShapedArray(shape, dtype))
            zero_outs.append(np.zeros(shape, dtype))
    n_params = len(in_names)
    all_in_names = list(in_names) + list(out_names)
    if partition_name is not None:
        all_in_names.append(partition_name)

    def _body(*args):
        operands = list(args)
        if partition_name is not None:
            operands.append(bass2jax.partition_id_tensor())
        outs = _bass_exec_p.bind(
            *operands,
            out_avals=tuple(out_avals),
            in_names=tuple(all_in_names),
            out_names=tuple(out_names),
            lowering_input_output_aliases=(),
            sim_require_finite=True,
            sim_require_nnan=True,
            nc=nc,
        )
        return tuple(outs)

    devices = jax.devices()[:NCORES]
    mesh = Mesh(np.asarray(devices), ("core",))
    in_specs = (PartitionSpec("core"),) * (n_params + len(out_names))
    out_specs = (PartitionSpec("core"),) * len(out_names)
    sharded = jax.jit(
        shard_map(_body, mesh=mesh, in_specs=in_specs, out_specs=out_specs,
                  check_rep=False),
        keep_unused=True,
    )

    def run_fn(in_maps, iters=1):
        import time
        per_core = [[np.asarray(m[name]) for name in in_names]
                    for m in in_maps]
        concat_in = [np.concatenate([per_core[c][i] for c in range(NCORES)],
                                    axis=0)
                     for i in range(n_params)]
        concat_zeros = [np.zeros((NCORES * z.shape[0], *z.shape[1:]), z.dtype)
                        for z in zero_outs]
        dev_in = [jax.device_put(a) for a in concat_in]
        dev_zero = [jax.device_put(z) for z in concat_zeros]
        out = sharded(*dev_in, *dev_zero)
        jax.block_until_ready(out)
        t0 = time.perf_counter()
        if iters > 1:
            for _ in range(iters):
                out = sharded(*dev_in, *dev_zero)
            jax.block_until_ready(out)
            dt = (time.perf_counter() - t0) / iters
        else:
            dt = 0.0
        results = [
            {name: np.asarray(out[i]).reshape(NCORES, *out_avals[i].shape)[c]
             for i, name in enumerate(out_names)}
            for c in range(NCORES)
        ]
        return results, dt

    return run_fn


_CACHE = {}


def _get_runner(p, stage=99):
    key = (p.skey, stage)
    if key in _CACHE:
        return _CACHE[key]
    nc = _build(p, stage=stage)
    run_fn = _make_runner(nc)
    _CACHE[key] = run_fn
    return run_fn


def _in_maps(p):
    return [{"blob": p.blob[r]} for r in range(NCORES)]


def kernel(**inputs):
    p = _preprocess(**inputs)
    run_fn = _get_runner(p)
    in_maps = _in_maps(p)
    results, dt = run_fn(in_maps, iters=1)
    kernel.last_dt = dt
    kernel.run_fn = run_fn
    kernel.in_maps = in_maps
    outs = [results[r]["out"][:RN] for r in range(NCORES)]
    return np.concatenate(outs, axis=0).astype(np.float32)
